# revision 27
# baseline (speedup 1.0000x reference)
"""MeshMeanFlowNet block on 8 Trainium2 NeuronCores.

Sharding: data-parallel over B (one batch element per core), no collectives.
Activations are feature-major on device ([feature, token]); the attention
softmax is computed in the transposed layout S^T[j, i] (j = key token on
partitions) with the denominator coming from a ones-row appended to V.

Key structure vs a naive port:
  * AdaLN parameter path (SiLU->Linear on cond) is precomputed on HOST --
    the 4MB wada weight never touches the device.
  * The per-edge-type/per-head softmax bias is applied by ONE custom DVE
    instruction per (head, key-block): P = E * p_h(e) where E = exp(S),
    e = edge type as bf16, and p_h is the cubic polynomial interpolating
    exp(bias[e,h] - bias[0,h]) at e = 0..3 (the exp(-bias[0,h]) factor
    cancels in softmax normalization). No mask tiles, no logit adds.
  * exp runs on the scalar engine over head-pair-wide [128, 2048] PSUM.
  * Score matmuls for the two heads of a pair run concurrently in the PE
    array via row tiling (K=64 each, rows 0-63 / 64-127).
  * All weights travel as bf16; elementwise work is spread over
    vector/gpsimd/scalar engines.
"""

import sys

sys.path.insert(0, "/opt/trn_rl_repo")

import ml_dtypes
import numpy as np

B, V, D, H = 8, 1024, 512, 8
HD = D // H  # 64
NCORES = 8

_cache = {}
_OP = None
_ACT_PATCHED = False


def _patch_act_tables():
    """Keep Exp/Ln only in the combined natural_log_exp set so the
    compiler's table chooser stops thrashing between sets (each
    ACT_TABLE_LOAD blocks the scalar engine ~1.3us). Set indices are
    preserved; contents are only filtered."""
    global _ACT_PATCHED
    if _ACT_PATCHED:
        return
    import concourse.bacc as bacc_mod
    import concourse.hw_specs as hs
    from concourse import mybir

    orig = hs.get_activation_tables
    A = mybir.ActivationFunctionType

    def patched(arch):
        t = dict(orig(arch))
        for name in list(t):
            if name != "natural_log_exp_and_others":
                s = set(t[name])
                s.discard(A.Exp)
                s.discard(A.Ln)
                t[name] = s
        return t

    hs.get_activation_tables = patched
    bacc_mod.get_activation_tables = patched
    _ACT_PATCHED = True


def _register_dve_op():
    """Custom DVE op: out = in0 * (1 + in1*(s0 + in1*(s1 + in1*imm2)))."""
    global _OP
    if _OP is not None:
        return _OP
    from concourse import dve_ops
    from concourse.dve_spec import C0, C1, C2, Spec, Src0, Src1, lower
    from concourse.dve_uop import DveOpSpec

    name = "EDGE_SOFTMAX_SCALE_ANT"
    for o in dve_ops.OPS:
        if o.name == name:
            _OP = o
            return o

    body = Src0 + Src0 * (Src1 * (C0 + Src1 * (C1 + Src1 * C2)))
    spec = Spec(
        body=body,
        reference=lambda in0, in1, s0, s1, imm2: in0
        + in0 * (in1 * (s0 + in1 * (s1 + in1 * imm2))),
    )
    opcode = dve_ops._CUSTOM_DVE_ROW_BASE + len(dve_ops.OPS)
    shas = {}
    for ver in ("v3",):
        compiled = DveOpSpec(
            name=name, opcode=opcode, uops=lower(spec, ver=ver), rd1_en=True
        )
        shas[ver] = compiled.sha(ver)
    op = dve_ops.DveOp(name, spec, subdim=False, uops_sha=shas)
    dve_ops.OPS.append(op)
    dve_ops._SUB_OPCODE_FOR_NAME[name] = opcode
    dve_ops.CUSTOM_DVE_SPECS[name] = spec
    _OP = op
    return op


def _build_program(qc):
    """qc: [H][3] cubic coefficients for the per-head edge-bias polynomial."""
    import contextlib

    import concourse.bacc as bacc
    import concourse.tile as tile
    from concourse import mybir

    op = _register_dve_op()
    _patch_act_tables()

    f32 = mybir.dt.float32
    f32r = mybir.dt.float32r
    bf16 = mybir.dt.bfloat16
    f8 = mybir.dt.float8e4
    DR = mybir.MatmulPerfMode.DoubleRow
    ALU = mybir.AluOpType
    ACTF = mybir.ActivationFunctionType

    nc = bacc.Bacc("TRN2", target_bir_lowering=False, debug=False,
                   num_devices=NCORES)

    # ---- DRAM I/O (per-core shard, host pre-laid-out) ----
    xT = nc.dram_tensor("xT", [D, V], f32r, kind="ExternalInput")
    eiT = nc.dram_tensor("eiT", [V, V], bf16, kind="ExternalInput")  # [j, i]
    wqk = nc.dram_tensor("wqk", [256, 2048], f8, kind="ExternalInput")
    wv = nc.dram_tensor("wv", [256, 1024], f8, kind="ExternalInput")
    wproj = nc.dram_tensor("wproj", [256, 1024], f8, kind="ExternalInput")
    wm1 = nc.dram_tensor("wm1", [256, 4096], f8, kind="ExternalInput")
    wm2 = nc.dram_tensor("wm2", [2048, D], bf16, kind="ExternalInput")
    biasd = nc.dram_tensor("biasd", [128, 32], f32, kind="ExternalInput")
    cvbd = nc.dram_tensor("cvbd", [128, 520], bf16, kind="ExternalInput")
    onesf = nc.dram_tensor("onesf", [128, 1], f32r, kind="ExternalInput")
    onesb = nc.dram_tensor("onesb", [128, 8], bf16, kind="ExternalInput")
    yT = nc.dram_tensor("yT", [D, V], f32, kind="ExternalOutput")

    def mm(out, lhsT, rhs, **kw):
        nc.tensor.matmul(out, lhsT.bitcast(f32r), rhs.bitcast(f32r), **kw)

    def mmb(out, lhsT, rhs, **kw):
        nc.tensor.matmul(out, lhsT, rhs, **kw)

    with tile.TileContext(nc) as tc:
        with contextlib.ExitStack() as ctx:
            persist = ctx.enter_context(tc.tile_pool(name="persist", bufs=1))

            # x first (critical path), then weights in consumption order;
            # DMA triggers dispatch serially on the sync queue (~0.65us
            # each) so fewer, bigger transfers matter.
            xT_t = [persist.tile([128, V], f32r, tag=f"xT{kc}",
                                 name=f"xT_t{kc}") for kc in range(4)]
            for half in range(2):
                sh = slice(half * 512, half * 512 + 512)
                for kc in range(4):
                    nc.sync.dma_start(out=xT_t[kc][:, sh],
                                      in_=xT[kc * 128:(kc + 1) * 128, sh])

            wqk_t = [persist.tile([128, 2, 1024], f8, tag=f"wqk{kc}",
                                  name="wqk_t") for kc in range(2)]
            wv_t = [persist.tile([128, 2, 512], f8, tag=f"wv{kc}",
                                 name="wv_t") for kc in range(2)]
            for kc in range(2):
                nc.sync.dma_start(
                    out=wqk_t[kc],
                    in_=wqk[kc * 128:(kc + 1) * 128, :].rearrange(
                        "p (ko n) -> p ko n", ko=2))
            for kc in range(2):
                nc.sync.dma_start(
                    out=wv_t[kc],
                    in_=wv[kc * 128:(kc + 1) * 128, :].rearrange(
                        "p (ko n) -> p ko n", ko=2))

            eit = [persist.tile([128, 2048], bf16, tag=f"eit{jp}",
                                name=f"eit{jp}") for jp in range(4)]
            for jp in range(4):
                for sel in range(2):
                    j0 = jp * 256 + sel * 128
                    nc.sync.dma_start(out=eit[jp][:, sel * 1024:
                                                  sel * 1024 + 1024],
                                      in_=eiT[j0:j0 + 128, :])

            biases = persist.tile([128, 32], f32, tag="biases")
            nc.sync.dma_start(out=biases, in_=biasd[:, :])
            bp_t = biases[:, 0:4]
            bm1_t = biases[:, 4:20]
            bm2_t = biases[:, 20:24]
            cqk = biases[:, 24:32]
            cvb = persist.tile([128, 8, 65], bf16, tag="cvb")
            nc.sync.dma_start(out=cvb,
                              in_=cvbd[:].rearrange("p (h c) -> p h c", h=8))
            ones = persist.tile([128, 1], f32r, tag="ones")
            nc.sync.dma_start(out=ones, in_=onesf[:, :])
            epst = persist.tile([1, 1], f32, tag="eps")
            nc.vector.memset(epst, 1e-5)

            wp_t = [persist.tile([128, 2, 512], f8, tag=f"wproj{kc}",
                                 name="wp_t") for kc in range(2)]
            wm1_t = [persist.tile([128, 2, 2048], f8, tag=f"wm1{kc}",
                                  name="wm1_t") for kc in range(2)]
            for kc in range(2):
                nc.sync.dma_start(
                    out=wp_t[kc],
                    in_=wproj[kc * 128:(kc + 1) * 128, :].rearrange(
                        "p (ko n) -> p ko n", ko=2))
            for kc in range(2):
                nc.sync.dma_start(
                    out=wm1_t[kc],
                    in_=wm1[kc * 128:(kc + 1) * 128, :].rearrange(
                        "p (ko n) -> p ko n", ko=2))
            wm2_t = persist.tile([128, 16, 512], bf16, tag="wm2",
                                 name="wm2_t")
            nc.sync.dma_start(
                out=wm2_t,
                in_=wm2[:].rearrange("(kc p) n -> p kc n", kc=16))

            x2 = [persist.tile([128, V], f32r, tag=f"x2_{kc}",
                               name=f"x2_{kc}") for kc in range(4)]
            att = [persist.tile([128, 2, V], f8, tag=f"att{g}",
                                name=f"att{g}") for g in range(2)]

            vec_or_gp = [nc.vector, nc.gpsimd]

            def adaln(src_tiles, dst_pool, out_tag, warm_after,
                      pair_fp8=False):
                """LayerNorm over features (partitions), computed per
                512-token half so downstream matmuls start on half 0 while
                half 1 is still in the stats chain. The adaptive affine is
                folded into host-scaled consumer weights."""
                if pair_fp8:
                    out = [dst_pool.tile([128, 2, V], f8,
                                         tag=f"{out_tag}{i}",
                                         name=f"ln_{out_tag}{i}")
                           for i in range(2)]
                else:
                    out = [dst_pool.tile([128, V], bf16,
                                         tag=f"{out_tag}{kc}",
                                         name=f"ln_{out_tag}{kc}")
                           for kc in range(4)]
                with tc.tile_pool(name="lnt", bufs=1) as lnt, \
                        tc.tile_pool(name="lnp", bufs=1, space="PSUM") as lnp:
                    for half in range(2):
                        sh = slice(half * 512, half * 512 + 512)
                        ps_s = lnp.tile([1, 512], f32, tag=f"lnsum{half}")
                        ps_q = lnp.tile([1, 512], f32, tag=f"lnsq{half}")
                        for kc in range(4):
                            sq = lnt.tile([128, 512], f32r, tag="lnsq",
                                          bufs=2, name="sq")
                            nc.scalar.square(
                                sq, src_tiles[kc].bitcast(f32)[:, sh])
                            mm(ps_s, ones, src_tiles[kc][:, sh],
                               start=(kc == 0), stop=(kc == 3))
                            mm(ps_q, ones, sq,
                               start=(kc == 0), stop=(kc == 3))
                        wk = lnp.tile([1, 512], f32, tag=f"wk{half}")
                        for w in range(16):
                            mm(wk, ones, warm_after[:, 0:512],
                               start=(w == 0), stop=(w == 15))
                        mean = lnt.tile([1, 512], f32, tag=f"mean{half}")
                        nc.scalar.mul(mean, ps_s, 1.0 / D)
                        msq = lnt.tile([1, 512], f32, tag=f"msq{half}")
                        nc.vector.tensor_mul(msq, mean, mean)
                        var = lnt.tile([1, 512], f32, tag=f"var{half}")
                        nc.vector.scalar_tensor_tensor(
                            var, ps_q, 1.0 / D, msq, ALU.mult,
                            ALU.subtract)
                        lnv = lnt.tile([1, 512], f32, tag=f"lnv{half}")
                        nc.scalar.activation(lnv, var, ACTF.Ln, bias=epst)
                        r = lnt.tile([1, 512], f32, tag=f"r{half}")
                        nc.scalar.activation(r, lnv, ACTF.Exp, scale=-0.5)
                        mr = lnt.tile([1, 512], f32, tag=f"mr{half}")
                        nc.vector.tensor_mul(mr, mean, r)
                        rb = lnt.tile([128, 512], f32, tag=f"rb{half}")
                        nc.gpsimd.partition_broadcast(rb, r)
                        mrb = lnt.tile([128, 512], f32, tag=f"mrb{half}")
                        nc.gpsimd.partition_broadcast(mrb, mr)
                        for kc in range(4):
                            u = lnt.tile([128, 512], f32, tag="lnu",
                                         bufs=2, name="u")
                            nc.vector.tensor_mul(
                                u, src_tiles[kc].bitcast(f32)[:, sh], rb)
                            if pair_fp8:
                                dst = out[kc // 2][:, kc % 2, sh]
                            else:
                                dst = out[kc][:, sh]
                            nc.vector.scalar_tensor_tensor(
                                dst, mrb, -1.0, u, ALU.mult, ALU.add)
                return out

            # ---- attention lifetime ----
            with tc.tile_pool(name="attlife", bufs=1) as attlife:
                qk = [attlife.tile([128, V], bf16, tag=f"qk{m}",
                                   name=f"qk{m}") for m in range(8)]
                vaug = [attlife.tile([128, 2, 8, 72], f8, tag=f"vaug{tp}",
                                     name=f"vaug{tp}") for tp in range(4)]
                for tp in range(4):
                    nc.vector.memset(vaug[tp][:, :, :, 64:65], 1.0)

                # h1 = AdaLN1(x); qk feature-major; v token-major
                with tc.tile_pool(name="h1pool", bufs=1) as h1pool:
                    h1 = adaln(xT_t, h1pool, "h1", xT_t[0], pair_fp8=True)
                    with tc.tile_pool(name="qkvp", bufs=4,
                                      space="PSUM") as qkvp:
                        cp_rot = 0
                        for nh in range(2):
                            s = slice(nh * 512, nh * 512 + 512)
                            for m in (0, 4, 1, 5, 2, 6, 3, 7):
                                pp = qkvp.tile([128, 512], f32, tag="mmqk")
                                for kc in range(2):
                                    mmb(pp,
                                        wqk_t[kc][:, :,
                                                  m * 128:(m + 1) * 128],
                                        h1[kc][:, :, s], start=(kc == 0),
                                        stop=(kc == 1), perf_mode=DR)
                                if cp_rot % 2 == 1:
                                    nc.scalar.activation(
                                        qk[m][:, s], pp, ACTF.Identity,
                                        bias=cqk[:, m:m + 1])
                                else:
                                    nc.vector.tensor_scalar(
                                        qk[m][:, s], pp, 1.0,
                                        cqk[:, m:m + 1], ALU.mult, ALU.add)
                                cp_rot += 1
                            for t in range(nh * 4, nh * 4 + 4):
                                pp = qkvp.tile([128, 512], f32, tag="mmv")
                                for kc in range(2):
                                    mmb(pp,
                                        h1[kc][:, :,
                                               t * 128:(t + 1) * 128],
                                        wv_t[kc], start=(kc == 0),
                                        stop=(kc == 1), perf_mode=DR)
                                nc.vector.tensor_add(
                                    vaug[t // 2][:, t % 2, :, 0:64],
                                    pp[:].rearrange("p (h d) -> p h d", h=8),
                                    cvb[:, :, 0:64])
                # attention core
                with tc.tile_pool(name="attt", bufs=1) as attt, \
                        tc.tile_pool(name="attps", bufs=1,
                                     space="PSUM") as attps:
                    ops = [attps.tile([65, V], f32, tag=f"ops{i}",
                                      name=f"ops{i}") for i in range(2)]
                    for hg in range(4):
                        kt = qk[4 + hg]
                        qt = qk[hg]
                        for jp in range(4):
                            P2 = [attt.tile([128, 2, V], f8,
                                            tag=f"P2_{hi}", bufs=3,
                                            name=f"P2_{hi}")
                                  for hi in range(2)]
                            Ep = [attt.tile([128, 2048], bf16,
                                            tag=f"Ep{hi}", bufs=3,
                                            name=f"Ep{hi}")
                                  for hi in range(2)]
                            for sel in range(2):
                                jt = jp * 2 + sel
                                jsl = slice(jt * 128, jt * 128 + 128)
                                S = [attps.tile([128, V], f32, tag="S",
                                                bufs=2, name=f"S{hi}")
                                     for hi in range(2)]
                                for nh in range(2):
                                    s = slice(nh * 512, nh * 512 + 512)
                                    mmb(S[0][:, s], kt[0:64, jsl],
                                        qt[0:64, s], start=True, stop=True,
                                        tile_position=(0, 0))
                                    mmb(S[1][:, s], kt[64:128, jsl],
                                        qt[64:128, s], start=True,
                                        stop=True, tile_position=(64, 0))
                                for hi in range(2):
                                    nc.scalar.activation(
                                        Ep[hi][:, sel * 1024:
                                               sel * 1024 + 1024],
                                        S[hi], ACTF.Exp, scale=0.125)
                            for hi in range(2):
                                h = hg * 2 + hi
                                nc.vector._custom_dve(
                                    op, out=P2[hi], in0=Ep[hi],
                                    in1=eit[jp],
                                    s0=qc[h][0], s1=qc[h][1],
                                    imm2=qc[h][2])
                                for nh in range(2):
                                    s = slice(nh * 512, nh * 512 + 512)
                                    mmb(ops[hi][:, s],
                                        vaug[jp][:, :, h, 0:65],
                                        P2[hi][:, :, s],
                                        start=(jp == 0),
                                        stop=(jp == 3), perf_mode=DR)
                        # normalize the head pair: copy numerators to
                        # SBUF (frees PSUM), 1/den via exp(-ln(den)) on the
                        # scalar engine, broadcast+multiply on gpsimd.
                        for hi in range(2):
                            dln = attt.tile([1, V], f32, tag=f"dln{hi}",
                                            bufs=2, name="dln")
                            nc.scalar.activation(dln, ops[hi][64:65, :],
                                                 ACTF.Ln)
                            rl = attt.tile([1, V], f32, tag=f"rl{hi}",
                                           bufs=2, name="rl")
                            nc.scalar.activation(rl, dln, ACTF.Exp,
                                                 scale=-1.0)
                            rlb = attt.tile([64, V], f32, tag=f"rlb{hi}",
                                            bufs=2, name="rlb")
                            nc.gpsimd.partition_broadcast(rlb, rl)
                            nc.vector.tensor_mul(
                                att[hg // 2][hi * 64:hi * 64 + 64,
                                             hg % 2, :],
                                ops[hi][0:64, :], rlb)
                # proj + residual -> x2
                with tc.tile_pool(name="projp", bufs=4,
                                  space="PSUM") as projp:
                    for nh in range(2):
                        for m in range(4):
                            s = slice(nh * 512, nh * 512 + 512)
                            pp = projp.tile([128, 512], f32, tag="mmproj")
                            for kc in range(2):
                                mmb(pp,
                                    wp_t[kc][:, :, m * 128:(m + 1) * 128],
                                    att[kc][:, :, s], start=(kc == 0),
                                    stop=(kc == 1), perf_mode=DR)
                            nc.vector.scalar_tensor_tensor(
                                x2[m][:, s], pp, bp_t[:, m:m + 1],
                                xT_t[m][:, s].bitcast(f32), ALU.add,
                                ALU.add)

            # ---- MLP branch (fp8 DoubleRow matmuls) ----
            with tc.tile_pool(name="mlplife", bufs=1) as mlplife:
                h2 = adaln(x2, mlplife, "h2", x2[0], pair_fp8=True)
                with tc.tile_pool(name="mlpt", bufs=1) as mlpt, \
                        tc.tile_pool(name="mlpp", bufs=4,
                                     space="PSUM") as mlpp:
                    for nh in range(2):
                        s = slice(nh * 512, nh * 512 + 512)
                        g = [mlpt.tile([128, 512], bf16, tag=f"g{m}",
                                       name=f"g{m}") for m in range(16)]
                        for m in range(16):
                            pp = mlpp.tile([128, 512], f32, tag="mmm1")
                            for i in range(2):
                                mmb(pp,
                                    wm1_t[i][:, :, m * 128:(m + 1) * 128],
                                    h2[i][:, :, s], start=(i == 0),
                                    stop=(i == 1), perf_mode=DR)
                            nc.scalar.activation(g[m], pp, ACTF.Gelu,
                                                 bias=bm1_t[:, m:m + 1])
                        for m in range(4):
                            pp = mlpp.tile([128, 512], f32, tag="mmm2")
                            for kc in range(16):
                                mmb(pp,
                                    wm2_t[:, kc, m * 128:(m + 1) * 128],
                                    g[kc], start=(kc == 0), stop=(kc == 15))
                            yt = mlpt.tile([128, 512], f32, tag="yt",
                                           bufs=2, name="yt")
                            nc.vector.scalar_tensor_tensor(
                                yt, pp, bm2_t[:, m:m + 1],
                                x2[m][:, s].bitcast(f32), ALU.add, ALU.add)
                            nc.sync.dma_start(
                                out=yT[m * 128:(m + 1) * 128, s], in_=yt)

    nc.compile()
    return nc


def _poly_coeffs(edge_table):
    """Per-head cubic p(e) = 1 + e*(q0 + e*(q1 + e*q2)) with
    p(e) = exp(table[e,h] - table[0,h]) for e = 0..3."""
    et = np.asarray(edge_table, dtype=np.float64)
    A = np.array([[1.0, 1.0, 1.0],
                  [1.0, 2.0, 4.0],
                  [1.0, 3.0, 9.0]])
    qc = []
    for h in range(H):
        g = np.exp(et[:, h] - et[0, h])
        rhs = np.array([(g[1] - 1.0) / 1.0,
                        (g[2] - 1.0) / 2.0,
                        (g[3] - 1.0) / 3.0])
        q = np.linalg.solve(A, rhs)
        # verify interpolation
        e = np.arange(4.0)
        p = 1.0 + e * (q[0] + e * (q[1] + e * q[2]))
        assert np.abs(p - g).max() < 1e-9
        qc.append([float(v) for v in q])
    return qc


def _silu(v):
    return v / (1.0 + np.exp(-v))


def _make_in_maps(inputs):
    x = np.asarray(inputs["x"], dtype=np.float32)
    cond = np.asarray(inputs["cond"], dtype=np.float32)
    ei = np.asarray(inputs["edge_index"])
    w_qkv = np.asarray(inputs["w_qkv"], dtype=np.float32)

    wqk = w_qkv[:, :2 * D].copy()  # 1/sqrt(hd) is applied inside exp
    wv = np.ascontiguousarray(w_qkv[:, 2 * D:])
    wm1 = np.asarray(inputs["w_mlp1"], dtype=np.float32)
    bm1 = np.asarray(inputs["b_mlp1"], dtype=np.float32)
    wm2_dev = np.asarray(inputs["w_mlp2"],
                         dtype=np.float32).astype(ml_dtypes.bfloat16)

    # host-side AdaLN parameter path; the scale folds into the consumer
    # weights, the shift into their (per-partition) output biases.
    sc = _silu(cond)  # [B, Dc]
    p1 = sc @ np.asarray(inputs["w_ada1"], dtype=np.float32) \
        + np.asarray(inputs["b_ada1"], dtype=np.float32)  # [B, 2D]
    p2 = sc @ np.asarray(inputs["w_ada2"], dtype=np.float32) \
        + np.asarray(inputs["b_ada2"], dtype=np.float32)
    s1, t1 = 1.0 + p1[:, :D], p1[:, D:]  # [B, D] each
    s2, t2 = 1.0 + p2[:, :D], p2[:, D:]

    shared = {
        "onesf": np.ones((128, 1), dtype=np.float32),
        "onesb": np.ones((128, 8), dtype=ml_dtypes.bfloat16),
        "wproj": np.ascontiguousarray(
            np.asarray(inputs["w_proj"], dtype=np.float32)
            .reshape(2, 2, 128, 512).transpose(0, 2, 1, 3)
            .reshape(256, 1024)).astype(ml_dtypes.float8_e4m3fn),

        "wm2": wm2_dev,
    }
    in_maps = []
    for b in range(B):
        cqk = wqk.T @ t1[b]                      # [1024]
        cv = wv.T @ t1[b]                        # [512]
        cvb = np.zeros((128, 8, 65), dtype=ml_dtypes.bfloat16)
        cvb[:, :, 0:64] = cv.reshape(8, 64).astype(ml_dtypes.bfloat16)
        cvb[:, :, 64] = ml_dtypes.bfloat16(1.0)
        bm1_b = bm1 + wm1.T @ t2[b]              # [2048]
        in_maps.append(dict(
            shared,
            xT=np.ascontiguousarray(x[b].T),
            eiT=np.ascontiguousarray(ei[b].T.astype(ml_dtypes.bfloat16)),
            wqk=np.ascontiguousarray(
                (wqk * s1[b][:, None]).reshape(2, 2, 128, 1024)
                .transpose(0, 2, 1, 3).reshape(256, 2048)
            ).astype(ml_dtypes.float8_e4m3fn),
            wv=np.ascontiguousarray(
                (wv * s1[b][:, None]).reshape(2, 2, 128, 512)
                .transpose(0, 2, 1, 3).reshape(256, 1024)
            ).astype(ml_dtypes.float8_e4m3fn),
            wm1=np.ascontiguousarray(
                (wm1 * s2[b][:, None]).reshape(2, 2, 128, 2048)
                .transpose(0, 2, 1, 3).reshape(256, 4096)
            ).astype(ml_dtypes.float8_e4m3fn),
            biasd=np.ascontiguousarray(np.concatenate([
                np.asarray(inputs["b_proj"],
                           dtype=np.float32).reshape(4, 128).T,
                bm1_b.reshape(16, 128).T,
                np.asarray(inputs["b_mlp2"],
                           dtype=np.float32).reshape(4, 128).T,
                cqk.reshape(8, 128).T,
            ], axis=1)),
            cvbd=np.ascontiguousarray(cvb.reshape(128, 520)),
        ))
    return in_maps


def kernel(**inputs):
    from concourse.bass_utils import run_bass_kernel_spmd

    et = np.asarray(inputs["edge_table"], dtype=np.float32)
    qc = _poly_coeffs(et)

    key = (et.tobytes(),)
    if key not in _cache:
        _cache[key] = _build_program(qc)
    nc = _cache[key]

    in_maps = _make_in_maps(inputs)
    res = run_bass_kernel_spmd(nc, in_maps, core_ids=list(range(NCORES)))
    out = np.stack([np.ascontiguousarray(res.results[b]["yT"].T)
                    for b in range(B)])
    return out.astype(np.float32)


# revision 28
# speedup vs baseline: 1.0287x; 1.0287x over previous
"""MeshMeanFlowNet block on 8 Trainium2 NeuronCores.

Sharding: data-parallel over B (one batch element per core), no collectives.
Activations are feature-major on device ([feature, token]); the attention
softmax is computed in the transposed layout S^T[j, i] (j = key token on
partitions) with the denominator coming from a ones-row appended to V.

Key structure vs a naive port:
  * AdaLN parameter path (SiLU->Linear on cond) is precomputed on HOST --
    the 4MB wada weight never touches the device.
  * The per-edge-type/per-head softmax bias is applied by ONE custom DVE
    instruction per (head, key-block): P = E * p_h(e) where E = exp(S),
    e = edge type as bf16, and p_h is the cubic polynomial interpolating
    exp(bias[e,h] - bias[0,h]) at e = 0..3 (the exp(-bias[0,h]) factor
    cancels in softmax normalization). No mask tiles, no logit adds.
  * exp runs on the scalar engine over head-pair-wide [128, 2048] PSUM.
  * Score matmuls for the two heads of a pair run concurrently in the PE
    array via row tiling (K=64 each, rows 0-63 / 64-127).
  * All weights travel as bf16; elementwise work is spread over
    vector/gpsimd/scalar engines.
"""

import sys

sys.path.insert(0, "/opt/trn_rl_repo")

import ml_dtypes
import numpy as np

B, V, D, H = 8, 1024, 512, 8
HD = D // H  # 64
NCORES = 8

_cache = {}
_OP = None
_ACT_PATCHED = False


def _patch_act_tables():
    """Keep Exp/Ln only in the combined natural_log_exp set so the
    compiler's table chooser stops thrashing between sets (each
    ACT_TABLE_LOAD blocks the scalar engine ~1.3us). Set indices are
    preserved; contents are only filtered."""
    global _ACT_PATCHED
    if _ACT_PATCHED:
        return
    import concourse.bacc as bacc_mod
    import concourse.hw_specs as hs
    from concourse import mybir

    orig = hs.get_activation_tables
    A = mybir.ActivationFunctionType

    def patched(arch):
        t = dict(orig(arch))
        for name in list(t):
            if name != "natural_log_exp_and_others":
                s = set(t[name])
                s.discard(A.Exp)
                s.discard(A.Ln)
                t[name] = s
        return t

    hs.get_activation_tables = patched
    bacc_mod.get_activation_tables = patched
    _ACT_PATCHED = True


def _register_dve_op():
    """Custom DVE op: out = in0 * (1 + in1*(s0 + in1*(s1 + in1*imm2)))."""
    global _OP
    if _OP is not None:
        return _OP
    from concourse import dve_ops
    from concourse.dve_spec import C0, C1, C2, Spec, Src0, Src1, lower
    from concourse.dve_uop import DveOpSpec

    name = "EDGE_SOFTMAX_SCALE_ANT"
    for o in dve_ops.OPS:
        if o.name == name:
            _OP = o
            return o

    body = Src0 + Src0 * (Src1 * (C0 + Src1 * (C1 + Src1 * C2)))
    spec = Spec(
        body=body,
        reference=lambda in0, in1, s0, s1, imm2: in0
        + in0 * (in1 * (s0 + in1 * (s1 + in1 * imm2))),
    )
    opcode = dve_ops._CUSTOM_DVE_ROW_BASE + len(dve_ops.OPS)
    shas = {}
    for ver in ("v3",):
        compiled = DveOpSpec(
            name=name, opcode=opcode, uops=lower(spec, ver=ver), rd1_en=True
        )
        shas[ver] = compiled.sha(ver)
    op = dve_ops.DveOp(name, spec, subdim=False, uops_sha=shas)
    dve_ops.OPS.append(op)
    dve_ops._SUB_OPCODE_FOR_NAME[name] = opcode
    dve_ops.CUSTOM_DVE_SPECS[name] = spec
    _OP = op
    return op


def _build_program(qc):
    """qc: [H][3] cubic coefficients for the per-head edge-bias polynomial."""
    import contextlib

    import concourse.bacc as bacc
    import concourse.tile as tile
    from concourse import mybir

    op = _register_dve_op()
    _patch_act_tables()

    f32 = mybir.dt.float32
    f32r = mybir.dt.float32r
    bf16 = mybir.dt.bfloat16
    f8 = mybir.dt.float8e4
    DR = mybir.MatmulPerfMode.DoubleRow
    ALU = mybir.AluOpType
    ACTF = mybir.ActivationFunctionType

    nc = bacc.Bacc("TRN2", target_bir_lowering=False, debug=False,
                   num_devices=NCORES)

    # ---- DRAM I/O (per-core shard, host pre-laid-out) ----
    xT = nc.dram_tensor("xT", [D, V], f32r, kind="ExternalInput")
    eiT = nc.dram_tensor("eiT", [V, V], bf16, kind="ExternalInput")  # [j, i]
    wqk = nc.dram_tensor("wqk", [256, 2048], f8, kind="ExternalInput")
    wv = nc.dram_tensor("wv", [256, 1024], f8, kind="ExternalInput")
    wproj = nc.dram_tensor("wproj", [256, 1024], f8, kind="ExternalInput")
    wm1 = nc.dram_tensor("wm1", [256, 4096], f8, kind="ExternalInput")
    wm2 = nc.dram_tensor("wm2", [2048, D], bf16, kind="ExternalInput")
    biasd = nc.dram_tensor("biasd", [128, 32], f32, kind="ExternalInput")
    cvbd = nc.dram_tensor("cvbd", [128, 520], bf16, kind="ExternalInput")
    onesf = nc.dram_tensor("onesf", [128, 1], f32r, kind="ExternalInput")
    onesb = nc.dram_tensor("onesb", [128, 8], bf16, kind="ExternalInput")
    yT = nc.dram_tensor("yT", [D, V], f32, kind="ExternalOutput")

    def mm(out, lhsT, rhs, **kw):
        nc.tensor.matmul(out, lhsT.bitcast(f32r), rhs.bitcast(f32r), **kw)

    def mmb(out, lhsT, rhs, **kw):
        nc.tensor.matmul(out, lhsT, rhs, **kw)

    with tile.TileContext(nc) as tc:
        with contextlib.ExitStack() as ctx:
            persist = ctx.enter_context(tc.tile_pool(name="persist", bufs=1))

            # x first (critical path), then weights in consumption order;
            # DMA triggers dispatch serially on the sync queue (~0.65us
            # each) so fewer, bigger transfers matter.
            xT_t = [persist.tile([128, V], f32r, tag=f"xT{kc}",
                                 name=f"xT_t{kc}") for kc in range(4)]
            for half in range(2):
                sh = slice(half * 512, half * 512 + 512)
                for kc in range(4):
                    nc.sync.dma_start(out=xT_t[kc][:, sh],
                                      in_=xT[kc * 128:(kc + 1) * 128, sh])

            wqk_t = [persist.tile([128, 2, 1024], f8, tag=f"wqk{kc}",
                                  name="wqk_t") for kc in range(2)]
            wv_t = [persist.tile([128, 2, 512], f8, tag=f"wv{kc}",
                                 name="wv_t") for kc in range(2)]
            for kc in range(2):
                nc.sync.dma_start(
                    out=wqk_t[kc],
                    in_=wqk[kc * 128:(kc + 1) * 128, :].rearrange(
                        "p (ko n) -> p ko n", ko=2))
            for kc in range(2):
                nc.sync.dma_start(
                    out=wv_t[kc],
                    in_=wv[kc * 128:(kc + 1) * 128, :].rearrange(
                        "p (ko n) -> p ko n", ko=2))

            biases = persist.tile([128, 32], f32, tag="biases")
            nc.sync.dma_start(out=biases, in_=biasd[:, :])
            bp_t = biases[:, 0:4]
            bm1_t = biases[:, 4:20]
            bm2_t = biases[:, 20:24]
            cqk = biases[:, 24:32]
            cvb = persist.tile([128, 8, 65], bf16, tag="cvb")
            nc.sync.dma_start(out=cvb,
                              in_=cvbd[:].rearrange("p (h c) -> p h c", h=8))
            ones = persist.tile([128, 1], f32r, tag="ones")
            nc.sync.dma_start(out=ones, in_=onesf[:, :])
            epst = persist.tile([1, 1], f32, tag="eps")
            nc.vector.memset(epst, 1e-5)

            eit = [persist.tile([128, 2048], bf16, tag=f"eit{jp}",
                                name=f"eit{jp}") for jp in range(4)]
            for jp in range(4):
                for sel in range(2):
                    j0 = jp * 256 + sel * 128
                    nc.sync.dma_start(out=eit[jp][:, sel * 1024:
                                                  sel * 1024 + 1024],
                                      in_=eiT[j0:j0 + 128, :])

            wp_t = [persist.tile([128, 2, 512], f8, tag=f"wproj{kc}",
                                 name="wp_t") for kc in range(2)]
            wm1_t = [persist.tile([128, 2, 2048], f8, tag=f"wm1{kc}",
                                  name="wm1_t") for kc in range(2)]
            for kc in range(2):
                nc.sync.dma_start(
                    out=wp_t[kc],
                    in_=wproj[kc * 128:(kc + 1) * 128, :].rearrange(
                        "p (ko n) -> p ko n", ko=2))
            for kc in range(2):
                nc.sync.dma_start(
                    out=wm1_t[kc],
                    in_=wm1[kc * 128:(kc + 1) * 128, :].rearrange(
                        "p (ko n) -> p ko n", ko=2))
            wm2_t = persist.tile([128, 16, 512], bf16, tag="wm2",
                                 name="wm2_t")
            for kc in range(16):
                nc.sync.dma_start(out=wm2_t[:, kc, :],
                                  in_=wm2[kc * 128:(kc + 1) * 128, :])

            x2 = [persist.tile([128, V], f32r, tag=f"x2_{kc}",
                               name=f"x2_{kc}") for kc in range(4)]
            att = [persist.tile([128, 2, V], f8, tag=f"att{g}",
                                name=f"att{g}") for g in range(2)]

            vec_or_gp = [nc.vector, nc.gpsimd]

            def adaln(src_tiles, dst_pool, out_tag, warm_after,
                      pair_fp8=False):
                """LayerNorm over features (partitions), computed per
                512-token half so downstream matmuls start on half 0 while
                half 1 is still in the stats chain. The adaptive affine is
                folded into host-scaled consumer weights."""
                if pair_fp8:
                    out = [dst_pool.tile([128, 2, V], f8,
                                         tag=f"{out_tag}{i}",
                                         name=f"ln_{out_tag}{i}")
                           for i in range(2)]
                else:
                    out = [dst_pool.tile([128, V], bf16,
                                         tag=f"{out_tag}{kc}",
                                         name=f"ln_{out_tag}{kc}")
                           for kc in range(4)]
                with tc.tile_pool(name="lnt", bufs=1) as lnt, \
                        tc.tile_pool(name="lnp", bufs=1, space="PSUM") as lnp:
                    for half in range(2):
                        sh = slice(half * 512, half * 512 + 512)
                        ps_s = lnp.tile([1, 512], f32, tag=f"lnsum{half}")
                        ps_q = lnp.tile([1, 512], f32, tag=f"lnsq{half}")
                        for kc in range(4):
                            sq = lnt.tile([128, 512], f32r, tag="lnsq",
                                          bufs=2, name="sq")
                            nc.scalar.square(
                                sq, src_tiles[kc].bitcast(f32)[:, sh])
                            mm(ps_s, ones, src_tiles[kc][:, sh],
                               start=(kc == 0), stop=(kc == 3))
                            mm(ps_q, ones, sq,
                               start=(kc == 0), stop=(kc == 3))
                        wk = lnp.tile([1, 512], f32, tag=f"wk{half}")
                        for w in range(16):
                            mm(wk, ones, warm_after[:, 0:512],
                               start=(w == 0), stop=(w == 15))
                        mean = lnt.tile([1, 512], f32, tag=f"mean{half}")
                        nc.scalar.mul(mean, ps_s, 1.0 / D)
                        msq = lnt.tile([1, 512], f32, tag=f"msq{half}")
                        nc.vector.tensor_mul(msq, mean, mean)
                        var = lnt.tile([1, 512], f32, tag=f"var{half}")
                        nc.vector.scalar_tensor_tensor(
                            var, ps_q, 1.0 / D, msq, ALU.mult,
                            ALU.subtract)
                        lnv = lnt.tile([1, 512], f32, tag=f"lnv{half}")
                        nc.scalar.activation(lnv, var, ACTF.Ln, bias=epst)
                        r = lnt.tile([1, 512], f32, tag=f"r{half}")
                        nc.scalar.activation(r, lnv, ACTF.Exp, scale=-0.5)
                        mr = lnt.tile([1, 512], f32, tag=f"mr{half}")
                        nc.vector.tensor_mul(mr, mean, r)
                        rb = lnt.tile([128, 512], f32, tag=f"rb{half}")
                        nc.gpsimd.partition_broadcast(rb, r)
                        mrb = lnt.tile([128, 512], f32, tag=f"mrb{half}")
                        nc.gpsimd.partition_broadcast(mrb, mr)
                        for kc in range(4):
                            u = lnt.tile([128, 512], f32, tag="lnu",
                                         bufs=2, name="u")
                            nc.vector.tensor_mul(
                                u, src_tiles[kc].bitcast(f32)[:, sh], rb)
                            if pair_fp8:
                                dst = out[kc // 2][:, kc % 2, sh]
                            else:
                                dst = out[kc][:, sh]
                            nc.vector.scalar_tensor_tensor(
                                dst, mrb, -1.0, u, ALU.mult, ALU.add)
                return out

            # ---- attention lifetime ----
            with tc.tile_pool(name="attlife", bufs=1) as attlife:
                qk = [attlife.tile([128, V], bf16, tag=f"qk{m}",
                                   name=f"qk{m}") for m in range(8)]
                vaug = [attlife.tile([128, 2, 8, 72], f8, tag=f"vaug{tp}",
                                     name=f"vaug{tp}") for tp in range(4)]
                for tp in range(4):
                    nc.vector.memset(vaug[tp][:, :, :, 64:65], 1.0)

                # h1 = AdaLN1(x); qk feature-major; v token-major
                with tc.tile_pool(name="h1pool", bufs=1) as h1pool:
                    h1 = adaln(xT_t, h1pool, "h1", xT_t[0], pair_fp8=True)
                    with tc.tile_pool(name="qkvp", bufs=4,
                                      space="PSUM") as qkvp:
                        cp_rot = 0
                        for nh in range(2):
                            s = slice(nh * 512, nh * 512 + 512)
                            for m in (0, 4, 1, 5, 2, 6, 3, 7):
                                pp = qkvp.tile([128, 512], f32, tag="mmqk")
                                for kc in range(2):
                                    mmb(pp,
                                        wqk_t[kc][:, :,
                                                  m * 128:(m + 1) * 128],
                                        h1[kc][:, :, s], start=(kc == 0),
                                        stop=(kc == 1), perf_mode=DR)
                                if cp_rot % 2 == 1:
                                    nc.scalar.activation(
                                        qk[m][:, s], pp, ACTF.Identity,
                                        bias=cqk[:, m:m + 1])
                                else:
                                    nc.vector.tensor_scalar(
                                        qk[m][:, s], pp, 1.0,
                                        cqk[:, m:m + 1], ALU.mult, ALU.add)
                                cp_rot += 1
                            for t in range(nh * 4, nh * 4 + 4):
                                pp = qkvp.tile([128, 512], f32, tag="mmv")
                                for kc in range(2):
                                    mmb(pp,
                                        h1[kc][:, :,
                                               t * 128:(t + 1) * 128],
                                        wv_t[kc], start=(kc == 0),
                                        stop=(kc == 1), perf_mode=DR)
                                nc.vector.tensor_add(
                                    vaug[t // 2][:, t % 2, :, 0:64],
                                    pp[:].rearrange("p (h d) -> p h d", h=8),
                                    cvb[:, :, 0:64])
                # attention core
                with tc.tile_pool(name="attt", bufs=1) as attt, \
                        tc.tile_pool(name="attps", bufs=1,
                                     space="PSUM") as attps:
                    ops = [attps.tile([65, V], f32, tag=f"ops{i}",
                                      name=f"ops{i}") for i in range(2)]
                    for hg in range(4):
                        kt = qk[4 + hg]
                        qt = qk[hg]
                        for jp in range(4):
                            P2 = [attt.tile([128, 2, V], f8,
                                            tag=f"P2_{hi}", bufs=3,
                                            name=f"P2_{hi}")
                                  for hi in range(2)]
                            Ep = [attt.tile([128, 2048], bf16,
                                            tag=f"Ep{hi}", bufs=3,
                                            name=f"Ep{hi}")
                                  for hi in range(2)]
                            for sel in range(2):
                                jt = jp * 2 + sel
                                jsl = slice(jt * 128, jt * 128 + 128)
                                S = [attps.tile([128, V], f32, tag="S",
                                                bufs=2, name=f"S{hi}")
                                     for hi in range(2)]
                                for nh in range(2):
                                    s = slice(nh * 512, nh * 512 + 512)
                                    mmb(S[0][:, s], kt[0:64, jsl],
                                        qt[0:64, s], start=True, stop=True,
                                        tile_position=(0, 0))
                                    mmb(S[1][:, s], kt[64:128, jsl],
                                        qt[64:128, s], start=True,
                                        stop=True, tile_position=(64, 0))
                                for hi in range(2):
                                    nc.scalar.activation(
                                        Ep[hi][:, sel * 1024:
                                               sel * 1024 + 1024],
                                        S[hi], ACTF.Exp, scale=0.125)
                            for hi in range(2):
                                h = hg * 2 + hi
                                nc.vector._custom_dve(
                                    op, out=P2[hi], in0=Ep[hi],
                                    in1=eit[jp],
                                    s0=qc[h][0], s1=qc[h][1],
                                    imm2=qc[h][2])
                                for nh in range(2):
                                    s = slice(nh * 512, nh * 512 + 512)
                                    mmb(ops[hi][:, s],
                                        vaug[jp][:, :, h, 0:65],
                                        P2[hi][:, :, s],
                                        start=(jp == 0),
                                        stop=(jp == 3), perf_mode=DR)
                        # normalize the head pair: copy numerators to
                        # SBUF (frees PSUM), 1/den via exp(-ln(den)) on the
                        # scalar engine, broadcast+multiply on gpsimd.
                        for hi in range(2):
                            dln = attt.tile([1, V], f32, tag=f"dln{hi}",
                                            bufs=2, name="dln")
                            nc.scalar.activation(dln, ops[hi][64:65, :],
                                                 ACTF.Ln)
                            rl = attt.tile([1, V], f32, tag=f"rl{hi}",
                                           bufs=2, name="rl")
                            nc.scalar.activation(rl, dln, ACTF.Exp,
                                                 scale=-1.0)
                            rlb = attt.tile([64, V], f32, tag=f"rlb{hi}",
                                            bufs=2, name="rlb")
                            nc.gpsimd.partition_broadcast(rlb, rl)
                            nc.vector.tensor_mul(
                                att[hg // 2][hi * 64:hi * 64 + 64,
                                             hg % 2, :],
                                ops[hi][0:64, :], rlb)
                # proj + residual -> x2
                with tc.tile_pool(name="projp", bufs=4,
                                  space="PSUM") as projp:
                    for nh in range(2):
                        for m in range(4):
                            s = slice(nh * 512, nh * 512 + 512)
                            pp = projp.tile([128, 512], f32, tag="mmproj")
                            for kc in range(2):
                                mmb(pp,
                                    wp_t[kc][:, :, m * 128:(m + 1) * 128],
                                    att[kc][:, :, s], start=(kc == 0),
                                    stop=(kc == 1), perf_mode=DR)
                            nc.vector.scalar_tensor_tensor(
                                x2[m][:, s], pp, bp_t[:, m:m + 1],
                                xT_t[m][:, s].bitcast(f32), ALU.add,
                                ALU.add)

            # ---- MLP branch (fp8 DoubleRow matmuls) ----
            with tc.tile_pool(name="mlplife", bufs=1) as mlplife:
                h2 = adaln(x2, mlplife, "h2", x2[0], pair_fp8=True)
                with tc.tile_pool(name="mlpt", bufs=1) as mlpt, \
                        tc.tile_pool(name="mlpp", bufs=4,
                                     space="PSUM") as mlpp:
                    for nh in range(2):
                        s = slice(nh * 512, nh * 512 + 512)
                        g = [mlpt.tile([128, 512], bf16, tag=f"g{m}",
                                       name=f"g{m}") for m in range(16)]
                        for m in range(16):
                            pp = mlpp.tile([128, 512], f32, tag="mmm1")
                            for i in range(2):
                                mmb(pp,
                                    wm1_t[i][:, :, m * 128:(m + 1) * 128],
                                    h2[i][:, :, s], start=(i == 0),
                                    stop=(i == 1), perf_mode=DR)
                            nc.scalar.activation(g[m], pp, ACTF.Gelu,
                                                 bias=bm1_t[:, m:m + 1])
                        for m in range(4):
                            pp = mlpp.tile([128, 512], f32, tag="mmm2")
                            for kc in range(16):
                                mmb(pp,
                                    wm2_t[:, kc, m * 128:(m + 1) * 128],
                                    g[kc], start=(kc == 0), stop=(kc == 15))
                            yt = mlpt.tile([128, 512], f32, tag="yt",
                                           bufs=2, name="yt")
                            nc.vector.scalar_tensor_tensor(
                                yt, pp, bm2_t[:, m:m + 1],
                                x2[m][:, s].bitcast(f32), ALU.add, ALU.add)
                            nc.sync.dma_start(
                                out=yT[m * 128:(m + 1) * 128, s], in_=yt)

    nc.compile()
    return nc


def _poly_coeffs(edge_table):
    """Per-head cubic p(e) = 1 + e*(q0 + e*(q1 + e*q2)) with
    p(e) = exp(table[e,h] - table[0,h]) for e = 0..3."""
    et = np.asarray(edge_table, dtype=np.float64)
    A = np.array([[1.0, 1.0, 1.0],
                  [1.0, 2.0, 4.0],
                  [1.0, 3.0, 9.0]])
    qc = []
    for h in range(H):
        g = np.exp(et[:, h] - et[0, h])
        rhs = np.array([(g[1] - 1.0) / 1.0,
                        (g[2] - 1.0) / 2.0,
                        (g[3] - 1.0) / 3.0])
        q = np.linalg.solve(A, rhs)
        # verify interpolation
        e = np.arange(4.0)
        p = 1.0 + e * (q[0] + e * (q[1] + e * q[2]))
        assert np.abs(p - g).max() < 1e-9
        qc.append([float(v) for v in q])
    return qc


def _silu(v):
    return v / (1.0 + np.exp(-v))


def _make_in_maps(inputs):
    x = np.asarray(inputs["x"], dtype=np.float32)
    cond = np.asarray(inputs["cond"], dtype=np.float32)
    ei = np.asarray(inputs["edge_index"])
    w_qkv = np.asarray(inputs["w_qkv"], dtype=np.float32)

    wqk = w_qkv[:, :2 * D].copy()  # 1/sqrt(hd) is applied inside exp
    wv = np.ascontiguousarray(w_qkv[:, 2 * D:])
    wm1 = np.asarray(inputs["w_mlp1"], dtype=np.float32)
    bm1 = np.asarray(inputs["b_mlp1"], dtype=np.float32)
    wm2_dev = np.asarray(inputs["w_mlp2"],
                         dtype=np.float32).astype(ml_dtypes.bfloat16)

    # host-side AdaLN parameter path; the scale folds into the consumer
    # weights, the shift into their (per-partition) output biases.
    sc = _silu(cond)  # [B, Dc]
    p1 = sc @ np.asarray(inputs["w_ada1"], dtype=np.float32) \
        + np.asarray(inputs["b_ada1"], dtype=np.float32)  # [B, 2D]
    p2 = sc @ np.asarray(inputs["w_ada2"], dtype=np.float32) \
        + np.asarray(inputs["b_ada2"], dtype=np.float32)
    s1, t1 = 1.0 + p1[:, :D], p1[:, D:]  # [B, D] each
    s2, t2 = 1.0 + p2[:, :D], p2[:, D:]

    shared = {
        "onesf": np.ones((128, 1), dtype=np.float32),
        "onesb": np.ones((128, 8), dtype=ml_dtypes.bfloat16),
        "wproj": np.ascontiguousarray(
            np.asarray(inputs["w_proj"], dtype=np.float32)
            .reshape(2, 2, 128, 512).transpose(0, 2, 1, 3)
            .reshape(256, 1024)).astype(ml_dtypes.float8_e4m3fn),

        "wm2": wm2_dev,
    }
    in_maps = []
    for b in range(B):
        cqk = wqk.T @ t1[b]                      # [1024]
        cv = wv.T @ t1[b]                        # [512]
        cvb = np.zeros((128, 8, 65), dtype=ml_dtypes.bfloat16)
        cvb[:, :, 0:64] = cv.reshape(8, 64).astype(ml_dtypes.bfloat16)
        cvb[:, :, 64] = ml_dtypes.bfloat16(1.0)
        bm1_b = bm1 + wm1.T @ t2[b]              # [2048]
        in_maps.append(dict(
            shared,
            xT=np.ascontiguousarray(x[b].T),
            eiT=np.ascontiguousarray(ei[b].T.astype(ml_dtypes.bfloat16)),
            wqk=np.ascontiguousarray(
                (wqk * s1[b][:, None]).reshape(2, 2, 128, 1024)
                .transpose(0, 2, 1, 3).reshape(256, 2048)
            ).astype(ml_dtypes.float8_e4m3fn),
            wv=np.ascontiguousarray(
                (wv * s1[b][:, None]).reshape(2, 2, 128, 512)
                .transpose(0, 2, 1, 3).reshape(256, 1024)
            ).astype(ml_dtypes.float8_e4m3fn),
            wm1=np.ascontiguousarray(
                (wm1 * s2[b][:, None]).reshape(2, 2, 128, 2048)
                .transpose(0, 2, 1, 3).reshape(256, 4096)
            ).astype(ml_dtypes.float8_e4m3fn),
            biasd=np.ascontiguousarray(np.concatenate([
                np.asarray(inputs["b_proj"],
                           dtype=np.float32).reshape(4, 128).T,
                bm1_b.reshape(16, 128).T,
                np.asarray(inputs["b_mlp2"],
                           dtype=np.float32).reshape(4, 128).T,
                cqk.reshape(8, 128).T,
            ], axis=1)),
            cvbd=np.ascontiguousarray(cvb.reshape(128, 520)),
        ))
    return in_maps


def kernel(**inputs):
    from concourse.bass_utils import run_bass_kernel_spmd

    et = np.asarray(inputs["edge_table"], dtype=np.float32)
    qc = _poly_coeffs(et)

    key = (et.tobytes(),)
    if key not in _cache:
        _cache[key] = _build_program(qc)
    nc = _cache[key]

    in_maps = _make_in_maps(inputs)
    res = run_bass_kernel_spmd(nc, in_maps, core_ids=list(range(NCORES)))
    out = np.stack([np.ascontiguousarray(res.results[b]["yT"].T)
                    for b in range(B)])
    return out.astype(np.float32)


# revision 29
# speedup vs baseline: 1.0786x; 1.0485x over previous
"""MeshMeanFlowNet block on 8 Trainium2 NeuronCores.

Sharding: data-parallel over B (one batch element per core), no collectives.
Activations are feature-major on device ([feature, token]); the attention
softmax is computed in the transposed layout S^T[j, i] (j = key token on
partitions) with the denominator coming from a ones-row appended to V.

Key structure vs a naive port:
  * AdaLN parameter path (SiLU->Linear on cond) is precomputed on HOST --
    the 4MB wada weight never touches the device.
  * The per-edge-type/per-head softmax bias is applied by ONE custom DVE
    instruction per (head, key-block): P = E * p_h(e) where E = exp(S),
    e = edge type as bf16, and p_h is the cubic polynomial interpolating
    exp(bias[e,h] - bias[0,h]) at e = 0..3 (the exp(-bias[0,h]) factor
    cancels in softmax normalization). No mask tiles, no logit adds.
  * exp runs on the scalar engine over head-pair-wide [128, 2048] PSUM.
  * Score matmuls for the two heads of a pair run concurrently in the PE
    array via row tiling (K=64 each, rows 0-63 / 64-127).
  * All weights travel as bf16; elementwise work is spread over
    vector/gpsimd/scalar engines.
"""

import sys

sys.path.insert(0, "/opt/trn_rl_repo")

import ml_dtypes
import numpy as np

B, V, D, H = 8, 1024, 512, 8
HD = D // H  # 64
NCORES = 8

_cache = {}
_OP = None
_ACT_PATCHED = False


def _patch_act_tables():
    """Keep Exp/Ln only in the combined natural_log_exp set so the
    compiler's table chooser stops thrashing between sets (each
    ACT_TABLE_LOAD blocks the scalar engine ~1.3us). Set indices are
    preserved; contents are only filtered."""
    global _ACT_PATCHED
    if _ACT_PATCHED:
        return
    import concourse.bacc as bacc_mod
    import concourse.hw_specs as hs
    from concourse import mybir

    orig = hs.get_activation_tables
    A = mybir.ActivationFunctionType

    def patched(arch):
        t = dict(orig(arch))
        for name in list(t):
            if name != "natural_log_exp_and_others":
                s = set(t[name])
                s.discard(A.Exp)
                s.discard(A.Ln)
                t[name] = s
        return t

    hs.get_activation_tables = patched
    bacc_mod.get_activation_tables = patched
    _ACT_PATCHED = True


def _register_dve_op():
    """Custom DVE op: out = in0 * (1 + in1*(s0 + in1*(s1 + in1*imm2)))."""
    global _OP
    if _OP is not None:
        return _OP
    from concourse import dve_ops
    from concourse.dve_spec import C0, C1, C2, Spec, Src0, Src1, lower
    from concourse.dve_uop import DveOpSpec

    name = "EDGE_SOFTMAX_SCALE_ANT"
    for o in dve_ops.OPS:
        if o.name == name:
            _OP = o
            return o

    body = Src0 + Src0 * (Src1 * (C0 + Src1 * (C1 + Src1 * C2)))
    spec = Spec(
        body=body,
        reference=lambda in0, in1, s0, s1, imm2: in0
        + in0 * (in1 * (s0 + in1 * (s1 + in1 * imm2))),
    )
    opcode = dve_ops._CUSTOM_DVE_ROW_BASE + len(dve_ops.OPS)
    shas = {}
    for ver in ("v3",):
        compiled = DveOpSpec(
            name=name, opcode=opcode, uops=lower(spec, ver=ver), rd1_en=True
        )
        shas[ver] = compiled.sha(ver)
    op = dve_ops.DveOp(name, spec, subdim=False, uops_sha=shas)
    dve_ops.OPS.append(op)
    dve_ops._SUB_OPCODE_FOR_NAME[name] = opcode
    dve_ops.CUSTOM_DVE_SPECS[name] = spec
    _OP = op
    return op


def _build_program(qc):
    """qc: [H][3] cubic coefficients for the per-head edge-bias polynomial."""
    import contextlib

    import concourse.bacc as bacc
    import concourse.tile as tile
    from concourse import mybir

    op = _register_dve_op()
    _patch_act_tables()

    f32 = mybir.dt.float32
    f32r = mybir.dt.float32r
    bf16 = mybir.dt.bfloat16
    f8 = mybir.dt.float8e4
    DR = mybir.MatmulPerfMode.DoubleRow
    ALU = mybir.AluOpType
    ACTF = mybir.ActivationFunctionType

    nc = bacc.Bacc("TRN2", target_bir_lowering=False, debug=False,
                   num_devices=NCORES)

    # ---- DRAM I/O (per-core shard, host pre-laid-out) ----
    xT = nc.dram_tensor("xT", [D, V], f32r, kind="ExternalInput")
    eiT = nc.dram_tensor("eiT", [V, V], bf16, kind="ExternalInput")  # [j, i]
    wqk = nc.dram_tensor("wqk", [256, 2048], f8, kind="ExternalInput")
    wv = nc.dram_tensor("wv", [256, 1024], f8, kind="ExternalInput")
    wproj = nc.dram_tensor("wproj", [256, 1024], f8, kind="ExternalInput")
    wm1 = nc.dram_tensor("wm1", [256, 4096], f8, kind="ExternalInput")
    wm2 = nc.dram_tensor("wm2", [1024, 1024], f8, kind="ExternalInput")
    biasd = nc.dram_tensor("biasd", [128, 32], f32, kind="ExternalInput")
    cvbd = nc.dram_tensor("cvbd", [128, 520], bf16, kind="ExternalInput")
    onesf = nc.dram_tensor("onesf", [128, 1], f32r, kind="ExternalInput")
    onesb = nc.dram_tensor("onesb", [128, 8], bf16, kind="ExternalInput")
    yT = nc.dram_tensor("yT", [D, V], f32, kind="ExternalOutput")

    def mm(out, lhsT, rhs, **kw):
        nc.tensor.matmul(out, lhsT.bitcast(f32r), rhs.bitcast(f32r), **kw)

    def mmb(out, lhsT, rhs, **kw):
        nc.tensor.matmul(out, lhsT, rhs, **kw)

    with tile.TileContext(nc) as tc:
        with contextlib.ExitStack() as ctx:
            persist = ctx.enter_context(tc.tile_pool(name="persist", bufs=1))

            # x first (critical path), then weights in consumption order;
            # DMA triggers dispatch serially on the sync queue (~0.65us
            # each) so fewer, bigger transfers matter.
            xT_t = [persist.tile([128, V], f32r, tag=f"xT{kc}",
                                 name=f"xT_t{kc}") for kc in range(4)]
            for half in range(2):
                sh = slice(half * 512, half * 512 + 512)
                for kc in range(4):
                    nc.sync.dma_start(out=xT_t[kc][:, sh],
                                      in_=xT[kc * 128:(kc + 1) * 128, sh])

            wqk_t = [persist.tile([128, 2, 1024], f8, tag=f"wqk{kc}",
                                  name="wqk_t") for kc in range(2)]
            wv_t = [persist.tile([128, 2, 512], f8, tag=f"wv{kc}",
                                 name="wv_t") for kc in range(2)]
            for kc in range(2):
                nc.sync.dma_start(
                    out=wqk_t[kc],
                    in_=wqk[kc * 128:(kc + 1) * 128, :].rearrange(
                        "p (ko n) -> p ko n", ko=2))
            for kc in range(2):
                nc.sync.dma_start(
                    out=wv_t[kc],
                    in_=wv[kc * 128:(kc + 1) * 128, :].rearrange(
                        "p (ko n) -> p ko n", ko=2))

            biases = persist.tile([128, 32], f32, tag="biases")
            nc.sync.dma_start(out=biases, in_=biasd[:, :])
            bp_t = biases[:, 0:4]
            bm1_t = biases[:, 4:20]
            bm2_t = biases[:, 20:24]
            cqk = biases[:, 24:32]
            cvb = persist.tile([128, 8, 65], bf16, tag="cvb")
            nc.sync.dma_start(out=cvb,
                              in_=cvbd[:].rearrange("p (h c) -> p h c", h=8))
            ones = persist.tile([128, 1], f32r, tag="ones")
            nc.sync.dma_start(out=ones, in_=onesf[:, :])
            epst = persist.tile([1, 1], f32, tag="eps")
            nc.vector.memset(epst, 1e-5)

            eit = [persist.tile([128, 2048], bf16, tag=f"eit{jp}",
                                name=f"eit{jp}") for jp in range(4)]
            for jp in range(4):
                for sel in range(2):
                    j0 = jp * 256 + sel * 128
                    nc.sync.dma_start(out=eit[jp][:, sel * 1024:
                                                  sel * 1024 + 1024],
                                      in_=eiT[j0:j0 + 128, :])

            wp_t = [persist.tile([128, 2, 512], f8, tag=f"wproj{kc}",
                                 name="wp_t") for kc in range(2)]
            wm1_t = [persist.tile([128, 2, 2048], f8, tag=f"wm1{kc}",
                                  name="wm1_t") for kc in range(2)]
            for kc in range(2):
                nc.sync.dma_start(
                    out=wp_t[kc],
                    in_=wproj[kc * 128:(kc + 1) * 128, :].rearrange(
                        "p (ko n) -> p ko n", ko=2))
            for kc in range(2):
                nc.sync.dma_start(
                    out=wm1_t[kc],
                    in_=wm1[kc * 128:(kc + 1) * 128, :].rearrange(
                        "p (ko n) -> p ko n", ko=2))
            wm2_t = persist.tile([128, 8, 2, 512], f8, tag="wm2",
                                 name="wm2_t")
            for mp in range(8):
                nc.sync.dma_start(
                    out=wm2_t[:, mp, :, :],
                    in_=wm2[mp * 128:(mp + 1) * 128, :].rearrange(
                        "p (ko n) -> p ko n", ko=2))

            x2 = [persist.tile([128, V], f32r, tag=f"x2_{kc}",
                               name=f"x2_{kc}") for kc in range(4)]
            att = [persist.tile([128, 2, V], f8, tag=f"att{g}",
                                name=f"att{g}") for g in range(2)]

            vec_or_gp = [nc.vector, nc.gpsimd]

            def adaln(src_tiles, dst_pool, out_tag, warm_after,
                      pair_fp8=False):
                """LayerNorm over features (partitions), computed per
                512-token half so downstream matmuls start on half 0 while
                half 1 is still in the stats chain. The adaptive affine is
                folded into host-scaled consumer weights."""
                if pair_fp8:
                    out = [dst_pool.tile([128, 2, V], f8,
                                         tag=f"{out_tag}{i}",
                                         name=f"ln_{out_tag}{i}")
                           for i in range(2)]
                else:
                    out = [dst_pool.tile([128, V], bf16,
                                         tag=f"{out_tag}{kc}",
                                         name=f"ln_{out_tag}{kc}")
                           for kc in range(4)]
                with tc.tile_pool(name="lnt", bufs=1) as lnt, \
                        tc.tile_pool(name="lnp", bufs=1, space="PSUM") as lnp:
                    for half in range(2):
                        sh = slice(half * 512, half * 512 + 512)
                        ps_s = lnp.tile([1, 512], f32, tag=f"lnsum{half}")
                        ps_q = lnp.tile([1, 512], f32, tag=f"lnsq{half}")
                        for kc in range(4):
                            sq = lnt.tile([128, 512], f32r, tag="lnsq",
                                          bufs=2, name="sq")
                            nc.scalar.square(
                                sq, src_tiles[kc].bitcast(f32)[:, sh])
                            mm(ps_s, ones, src_tiles[kc][:, sh],
                               start=(kc == 0), stop=(kc == 3))
                            mm(ps_q, ones, sq,
                               start=(kc == 0), stop=(kc == 3))
                        wk = lnp.tile([1, 512], f32, tag=f"wk{half}")
                        for w in range(16):
                            mm(wk, ones, warm_after[:, 0:512],
                               start=(w == 0), stop=(w == 15))
                        mean = lnt.tile([1, 512], f32, tag=f"mean{half}")
                        nc.scalar.mul(mean, ps_s, 1.0 / D)
                        msq = lnt.tile([1, 512], f32, tag=f"msq{half}")
                        nc.vector.tensor_mul(msq, mean, mean)
                        var = lnt.tile([1, 512], f32, tag=f"var{half}")
                        nc.vector.scalar_tensor_tensor(
                            var, ps_q, 1.0 / D, msq, ALU.mult,
                            ALU.subtract)
                        lnv = lnt.tile([1, 512], f32, tag=f"lnv{half}")
                        nc.scalar.activation(lnv, var, ACTF.Ln, bias=epst)
                        r = lnt.tile([1, 512], f32, tag=f"r{half}")
                        nc.scalar.activation(r, lnv, ACTF.Exp, scale=-0.5)
                        mr = lnt.tile([1, 512], f32, tag=f"mr{half}")
                        nc.vector.tensor_mul(mr, mean, r)
                        rb = lnt.tile([128, 512], f32, tag=f"rb{half}")
                        nc.gpsimd.partition_broadcast(rb, r)
                        mrb = lnt.tile([128, 512], f32, tag=f"mrb{half}")
                        nc.gpsimd.partition_broadcast(mrb, mr)
                        for kc in range(4):
                            u = lnt.tile([128, 512], f32, tag="lnu",
                                         bufs=2, name="u")
                            nc.vector.tensor_mul(
                                u, src_tiles[kc].bitcast(f32)[:, sh], rb)
                            if pair_fp8:
                                dst = out[kc // 2][:, kc % 2, sh]
                            else:
                                dst = out[kc][:, sh]
                            nc.vector.scalar_tensor_tensor(
                                dst, mrb, -1.0, u, ALU.mult, ALU.add)
                return out

            # ---- attention lifetime ----
            with tc.tile_pool(name="attlife", bufs=1) as attlife:
                qk = [attlife.tile([128, V], bf16, tag=f"qk{m}",
                                   name=f"qk{m}") for m in range(8)]
                vaug = [attlife.tile([128, 2, 8, 72], f8, tag=f"vaug{tp}",
                                     name=f"vaug{tp}") for tp in range(4)]
                for tp in range(4):
                    nc.vector.memset(vaug[tp][:, :, :, 64:65], 1.0)

                # h1 = AdaLN1(x); qk feature-major; v token-major
                with tc.tile_pool(name="h1pool", bufs=1) as h1pool:
                    h1 = adaln(xT_t, h1pool, "h1", xT_t[0], pair_fp8=True)
                    with tc.tile_pool(name="qkvp", bufs=4,
                                      space="PSUM") as qkvp:
                        cp_rot = 0
                        for nh in range(2):
                            s = slice(nh * 512, nh * 512 + 512)
                            for m in (0, 4, 1, 5, 2, 6, 3, 7):
                                pp = qkvp.tile([128, 512], f32, tag="mmqk")
                                for kc in range(2):
                                    mmb(pp,
                                        wqk_t[kc][:, :,
                                                  m * 128:(m + 1) * 128],
                                        h1[kc][:, :, s], start=(kc == 0),
                                        stop=(kc == 1), perf_mode=DR)
                                if cp_rot % 2 == 1:
                                    nc.scalar.activation(
                                        qk[m][:, s], pp, ACTF.Identity,
                                        bias=cqk[:, m:m + 1])
                                else:
                                    nc.vector.tensor_scalar(
                                        qk[m][:, s], pp, 1.0,
                                        cqk[:, m:m + 1], ALU.mult, ALU.add)
                                cp_rot += 1
                            for t in range(nh * 4, nh * 4 + 4):
                                pp = qkvp.tile([128, 512], f32, tag="mmv")
                                for kc in range(2):
                                    mmb(pp,
                                        h1[kc][:, :,
                                               t * 128:(t + 1) * 128],
                                        wv_t[kc], start=(kc == 0),
                                        stop=(kc == 1), perf_mode=DR)
                                nc.vector.tensor_add(
                                    vaug[t // 2][:, t % 2, :, 0:64],
                                    pp[:].rearrange("p (h d) -> p h d", h=8),
                                    cvb[:, :, 0:64])
                # attention core
                with tc.tile_pool(name="attt", bufs=1) as attt, \
                        tc.tile_pool(name="attps", bufs=1,
                                     space="PSUM") as attps:
                    ops = [attps.tile([65, V], f32, tag=f"ops{i}",
                                      name=f"ops{i}") for i in range(2)]
                    for hg in range(4):
                        kt = qk[4 + hg]
                        qt = qk[hg]
                        for jp in range(4):
                            P2 = [attt.tile([128, 2, V], f8,
                                            tag=f"P2_{hi}", bufs=3,
                                            name=f"P2_{hi}")
                                  for hi in range(2)]
                            Ep = [attt.tile([128, 2048], bf16,
                                            tag=f"Ep{hi}", bufs=3,
                                            name=f"Ep{hi}")
                                  for hi in range(2)]
                            for sel in range(2):
                                jt = jp * 2 + sel
                                jsl = slice(jt * 128, jt * 128 + 128)
                                S = [attps.tile([128, V], f32, tag="S",
                                                bufs=2, name=f"S{hi}")
                                     for hi in range(2)]
                                for nh in range(2):
                                    s = slice(nh * 512, nh * 512 + 512)
                                    mmb(S[0][:, s], kt[0:64, jsl],
                                        qt[0:64, s], start=True, stop=True,
                                        tile_position=(0, 0))
                                    mmb(S[1][:, s], kt[64:128, jsl],
                                        qt[64:128, s], start=True,
                                        stop=True, tile_position=(64, 0))
                                for hi in range(2):
                                    nc.scalar.activation(
                                        Ep[hi][:, sel * 1024:
                                               sel * 1024 + 1024],
                                        S[hi], ACTF.Exp, scale=0.125)
                            for hi in range(2):
                                h = hg * 2 + hi
                                nc.vector._custom_dve(
                                    op, out=P2[hi], in0=Ep[hi],
                                    in1=eit[jp],
                                    s0=qc[h][0], s1=qc[h][1],
                                    imm2=qc[h][2])
                                for nh in range(2):
                                    s = slice(nh * 512, nh * 512 + 512)
                                    mmb(ops[hi][:, s],
                                        vaug[jp][:, :, h, 0:65],
                                        P2[hi][:, :, s],
                                        start=(jp == 0),
                                        stop=(jp == 3), perf_mode=DR)
                        # normalize the head pair: copy numerators to
                        # SBUF (frees PSUM), 1/den via exp(-ln(den)) on the
                        # scalar engine, broadcast+multiply on gpsimd.
                        for hi in range(2):
                            dln = attt.tile([1, V], f32, tag=f"dln{hi}",
                                            bufs=2, name="dln")
                            nc.scalar.activation(dln, ops[hi][64:65, :],
                                                 ACTF.Ln)
                            rl = attt.tile([1, V], f32, tag=f"rl{hi}",
                                           bufs=2, name="rl")
                            nc.scalar.activation(rl, dln, ACTF.Exp,
                                                 scale=-1.0)
                            rlb = attt.tile([64, V], f32, tag=f"rlb{hi}",
                                            bufs=2, name="rlb")
                            nc.gpsimd.partition_broadcast(rlb, rl)
                            nc.vector.tensor_mul(
                                att[hg // 2][hi * 64:hi * 64 + 64,
                                             hg % 2, :],
                                ops[hi][0:64, :], rlb)
                # proj + residual -> x2
                with tc.tile_pool(name="projp", bufs=4,
                                  space="PSUM") as projp:
                    for nh in range(2):
                        for m in range(4):
                            s = slice(nh * 512, nh * 512 + 512)
                            pp = projp.tile([128, 512], f32, tag="mmproj")
                            for kc in range(2):
                                mmb(pp,
                                    wp_t[kc][:, :, m * 128:(m + 1) * 128],
                                    att[kc][:, :, s], start=(kc == 0),
                                    stop=(kc == 1), perf_mode=DR)
                            nc.vector.scalar_tensor_tensor(
                                x2[m][:, s], pp, bp_t[:, m:m + 1],
                                xT_t[m][:, s].bitcast(f32), ALU.add,
                                ALU.add)

            # ---- MLP branch (fp8 DoubleRow matmuls) ----
            with tc.tile_pool(name="mlplife", bufs=1) as mlplife:
                h2 = adaln(x2, mlplife, "h2", x2[0], pair_fp8=True)
                with tc.tile_pool(name="mlpt", bufs=1) as mlpt, \
                        tc.tile_pool(name="mlpp", bufs=4,
                                     space="PSUM") as mlpp:
                    for nh in range(2):
                        s = slice(nh * 512, nh * 512 + 512)
                        g = [mlpt.tile([128, 2, 512], f8, tag=f"g{mp}",
                                       name=f"g{mp}") for mp in range(8)]
                        for m in range(16):
                            pp = mlpp.tile([128, 512], f32, tag="mmm1")
                            for i in range(2):
                                mmb(pp,
                                    wm1_t[i][:, :, m * 128:(m + 1) * 128],
                                    h2[i][:, :, s], start=(i == 0),
                                    stop=(i == 1), perf_mode=DR)
                            nc.scalar.activation(g[m // 2][:, m % 2, :],
                                                 pp, ACTF.Gelu,
                                                 bias=bm1_t[:, m:m + 1])
                        for m in range(4):
                            pp = mlpp.tile([128, 512], f32, tag="mmm2")
                            for mp in range(8):
                                mmb(pp,
                                    wm2_t[:, mp, :,
                                          m * 128:(m + 1) * 128],
                                    g[mp], start=(mp == 0), stop=(mp == 7),
                                    perf_mode=DR)
                            yt = mlpt.tile([128, 512], f32, tag="yt",
                                           bufs=2, name="yt")
                            nc.vector.scalar_tensor_tensor(
                                yt, pp, bm2_t[:, m:m + 1],
                                x2[m][:, s].bitcast(f32), ALU.add, ALU.add)
                            nc.sync.dma_start(
                                out=yT[m * 128:(m + 1) * 128, s], in_=yt)

    nc.compile()
    return nc


def _poly_coeffs(edge_table):
    """Per-head cubic p(e) = 1 + e*(q0 + e*(q1 + e*q2)) with
    p(e) = exp(table[e,h] - table[0,h]) for e = 0..3."""
    et = np.asarray(edge_table, dtype=np.float64)
    A = np.array([[1.0, 1.0, 1.0],
                  [1.0, 2.0, 4.0],
                  [1.0, 3.0, 9.0]])
    qc = []
    for h in range(H):
        g = np.exp(et[:, h] - et[0, h])
        rhs = np.array([(g[1] - 1.0) / 1.0,
                        (g[2] - 1.0) / 2.0,
                        (g[3] - 1.0) / 3.0])
        q = np.linalg.solve(A, rhs)
        # verify interpolation
        e = np.arange(4.0)
        p = 1.0 + e * (q[0] + e * (q[1] + e * q[2]))
        assert np.abs(p - g).max() < 1e-9
        qc.append([float(v) for v in q])
    return qc


def _silu(v):
    return v / (1.0 + np.exp(-v))


def _make_in_maps(inputs):
    x = np.asarray(inputs["x"], dtype=np.float32)
    cond = np.asarray(inputs["cond"], dtype=np.float32)
    ei = np.asarray(inputs["edge_index"])
    w_qkv = np.asarray(inputs["w_qkv"], dtype=np.float32)

    wqk = w_qkv[:, :2 * D].copy()  # 1/sqrt(hd) is applied inside exp
    wv = np.ascontiguousarray(w_qkv[:, 2 * D:])
    wm1 = np.asarray(inputs["w_mlp1"], dtype=np.float32)
    bm1 = np.asarray(inputs["b_mlp1"], dtype=np.float32)
    wm2_dev = np.ascontiguousarray(
        np.asarray(inputs["w_mlp2"], dtype=np.float32)
        .reshape(8, 2, 128, 512).transpose(0, 2, 1, 3)
        .reshape(1024, 1024)).astype(ml_dtypes.float8_e4m3fn)

    # host-side AdaLN parameter path; the scale folds into the consumer
    # weights, the shift into their (per-partition) output biases.
    sc = _silu(cond)  # [B, Dc]
    p1 = sc @ np.asarray(inputs["w_ada1"], dtype=np.float32) \
        + np.asarray(inputs["b_ada1"], dtype=np.float32)  # [B, 2D]
    p2 = sc @ np.asarray(inputs["w_ada2"], dtype=np.float32) \
        + np.asarray(inputs["b_ada2"], dtype=np.float32)
    s1, t1 = 1.0 + p1[:, :D], p1[:, D:]  # [B, D] each
    s2, t2 = 1.0 + p2[:, :D], p2[:, D:]

    shared = {
        "onesf": np.ones((128, 1), dtype=np.float32),
        "onesb": np.ones((128, 8), dtype=ml_dtypes.bfloat16),
        "wproj": np.ascontiguousarray(
            np.asarray(inputs["w_proj"], dtype=np.float32)
            .reshape(2, 2, 128, 512).transpose(0, 2, 1, 3)
            .reshape(256, 1024)).astype(ml_dtypes.float8_e4m3fn),

        "wm2": wm2_dev,
    }
    in_maps = []
    for b in range(B):
        cqk = wqk.T @ t1[b]                      # [1024]
        cv = wv.T @ t1[b]                        # [512]
        cvb = np.zeros((128, 8, 65), dtype=ml_dtypes.bfloat16)
        cvb[:, :, 0:64] = cv.reshape(8, 64).astype(ml_dtypes.bfloat16)
        cvb[:, :, 64] = ml_dtypes.bfloat16(1.0)
        bm1_b = bm1 + wm1.T @ t2[b]              # [2048]
        in_maps.append(dict(
            shared,
            xT=np.ascontiguousarray(x[b].T),
            eiT=np.ascontiguousarray(ei[b].T.astype(ml_dtypes.bfloat16)),
            wqk=np.ascontiguousarray(
                (wqk * s1[b][:, None]).reshape(2, 2, 128, 1024)
                .transpose(0, 2, 1, 3).reshape(256, 2048)
            ).astype(ml_dtypes.float8_e4m3fn),
            wv=np.ascontiguousarray(
                (wv * s1[b][:, None]).reshape(2, 2, 128, 512)
                .transpose(0, 2, 1, 3).reshape(256, 1024)
            ).astype(ml_dtypes.float8_e4m3fn),
            wm1=np.ascontiguousarray(
                (wm1 * s2[b][:, None]).reshape(2, 2, 128, 2048)
                .transpose(0, 2, 1, 3).reshape(256, 4096)
            ).astype(ml_dtypes.float8_e4m3fn),
            biasd=np.ascontiguousarray(np.concatenate([
                np.asarray(inputs["b_proj"],
                           dtype=np.float32).reshape(4, 128).T,
                bm1_b.reshape(16, 128).T,
                np.asarray(inputs["b_mlp2"],
                           dtype=np.float32).reshape(4, 128).T,
                cqk.reshape(8, 128).T,
            ], axis=1)),
            cvbd=np.ascontiguousarray(cvb.reshape(128, 520)),
        ))
    return in_maps


def kernel(**inputs):
    from concourse.bass_utils import run_bass_kernel_spmd

    et = np.asarray(inputs["edge_table"], dtype=np.float32)
    qc = _poly_coeffs(et)

    key = (et.tobytes(),)
    if key not in _cache:
        _cache[key] = _build_program(qc)
    nc = _cache[key]

    in_maps = _make_in_maps(inputs)
    res = run_bass_kernel_spmd(nc, in_maps, core_ids=list(range(NCORES)))
    out = np.stack([np.ascontiguousarray(res.results[b]["yT"].T)
                    for b in range(B)])
    return out.astype(np.float32)
